# revision 35
# baseline (speedup 1.0000x reference)
"""GNN message-passing kernel for Trainium2 (8 NeuronCores, edge-parallel).

Strategy: LPT bin-pack nodes into 400 blocks (<=128 nodes, <=768 edges
each -> uniform 6 edge-chunks/block), snake-deal blocks to the 8 cores by
edge load. Each core owns its blocks' edges; outputs are disjoint -> no
collective; the host scatters block slots back to node ids.

Device pipeline per core (50 blocks x 6 chunks of 128 edges):
  1. PE: per-edge Q/K/V projections; bias comes free via a rank-1 PSUM
     seed (ones^T x bias-row, start=True) that the projection accumulates
     onto. K and V share lhsT=xt -> fused f=512 matmul.
  2. ACT: PSUM->SBUF fp16 casts (scalar engine, keeps DVE free).
  3. DVE: per-edge 8x8 head attention entirely in packed fp16 so every
     tensor_tensor hits the 2x_1p mode; d-sums and g-sums use TT add-trees
     (TENSOR_REDUCE is stuck at 1 elem/cycle; packed TT adds are 2x).
     exp on ACT (u stays f32 for range), reciprocal_approx_fast for 1/sum.
  4. PE: segment-sum via block one-hot matmul (S matrix from host),
     plus edge-type one-hot/count columns for the embedding & bias terms.
  5. PE: final output projection [Wo^T | emb@Wo^T | bo] per node block.
"""

import os
import sys

sys.path.insert(0, "/opt/trn_rl_repo")

import numpy as np
import ml_dtypes

from concourse import bass, bacc, mybir
import concourse.tile as tile
from concourse.bass_utils import run_bass_kernel_spmd

N_NODES = 50000
N_CORES = 8
IN_DIM = 128
HID = 256
H = 8
D = 32

BF16 = ml_dtypes.bfloat16
GP_MOD = 10**9     # GpSimd offload disabled: SBUF contention slows DVE
_prog_cache = {}
LAST_RESULTS = None


def _pack_blocks(deg, n_blocks, ecap, ncap=128):
    """LPT-pack nodes into n_blocks bins: <=ncap nodes, <=ecap edges each.
    Returns list of node-id lists, or None if infeasible."""
    import heapq
    order = np.argsort(-deg, kind="stable")
    heap = [(0, 0, b) for b in range(n_blocks)]
    heapq.heapify(heap)
    members = [[] for _ in range(n_blocks)]
    for n in order:
        d = int(deg[n])
        popped = []
        ok = False
        while heap:
            l, c, b = heapq.heappop(heap)
            if c < ncap and l + d <= ecap:
                members[b].append(n)
                heapq.heappush(heap, (l + d, c + 1, b))
                ok = True
                break
            popped.append((l, c, b))
        for p in popped:
            heapq.heappush(heap, p)
        if not ok:
            return None
    return members


def _build_program(NB, C):
    """C = edge-chunks (of 128) per node block; T = NB*C tiles per core."""
    NODES_PAD = NB * 128
    T = NB * C
    E_pad = T * 128
    f32, bf16 = mybir.dt.float32, mybir.dt.bfloat16
    f16 = mybir.dt.float16
    X = mybir.AxisListType.X
    MUL, ADD = mybir.AluOpType.mult, mybir.AluOpType.add

    nc = bacc.Bacc("TRN2", target_bir_lowering=False)
    xsT = nc.dram_tensor("xsT", [128, E_pad], bf16, kind="ExternalInput")
    xtT = nc.dram_tensor("xtT", [128, E_pad], bf16, kind="ExternalInput")
    S2 = nc.dram_tensor("S2", [128, T * 128], f16, kind="ExternalInput")
    OHt = nc.dram_tensor("OHt", [128, T * 4], f16, kind="ExternalInput")
    Wt = nc.dram_tensor("Wt", [128, 768], bf16, kind="ExternalInput")
    Brow = nc.dram_tensor("Brow", [1, 768], bf16, kind="ExternalInput")
    Ones = nc.dram_tensor("Ones", [1, 128], bf16, kind="ExternalInput")
    W2ab = nc.dram_tensor("W2ab", [128, 256], bf16, kind="ExternalInput")
    W2c = nc.dram_tensor("W2c", [4, 128], bf16, kind="ExternalInput")
    out = nc.dram_tensor("out", [128, NODES_PAD], f32, kind="ExternalOutput")

    with tile.TileContext(nc) as tc:
        with tc.tile_pool(name="const", bufs=1) as cp, \
             tc.tile_pool(name="io", bufs=2) as iop, \
             tc.tile_pool(name="work", bufs=2) as wp, \
             tc.tile_pool(name="pproj", bufs=1, space="PSUM") as pp, \
             tc.tile_pool(name="pacc", bufs=1, space="PSUM") as pa:

            wt = cp.tile([128, 768], bf16)
            nc.sync.dma_start(out=wt[:], in_=Wt[:, :])
            brow = cp.tile([1, 768], bf16)
            nc.sync.dma_start(out=brow[:], in_=Brow[:, :])
            ones = cp.tile([1, 128], bf16)
            nc.sync.dma_start(out=ones[:], in_=Ones[:, :])
            oh = cp.tile([128, T * 4], f16)
            nc.sync.dma_start(out=oh[:], in_=OHt[:, :])
            w2ab = cp.tile([128, 256], bf16)
            nc.sync.dma_start(out=w2ab[:], in_=W2ab[:, :])
            w2c = cp.tile([4, 128], bf16)
            nc.sync.dma_start(out=w2c[:], in_=W2c[:, :])
            outsb = cp.tile([128, NODES_PAD], f32)

            for b in range(NB):
                esl = slice(b * C * 128, (b + 1) * C * 128)
                xs = iop.tile([128, C * 128], bf16, tag="xs")
                nc.sync.dma_start(out=xs[:], in_=xsT[:, esl])
                xt = iop.tile([128, C * 128], bf16, tag="xt")
                nc.sync.dma_start(out=xt[:], in_=xtT[:, esl])
                sb = iop.tile([128, C * 128], f16, tag="sb")
                nc.sync.dma_start(out=sb[:], in_=S2[:, esl])

                qkv = wp.tile([128, C * 768], f16, tag="qkv")
                for i in range(C):
                    ps_q = pp.tile([128, 256], f32, tag="psq")
                    ps_kv = pp.tile([128, 512], f32, tag="pskv")
                    ei = slice(i * 128, (i + 1) * 128)
                    # bias via rank-1 PSUM seed (ones^T x brow), then the
                    # projection accumulates on top -> bias-add is free on
                    # PE; K and V share lhsT=xt so they fuse into one f=512
                    # matmul per stage
                    nc.tensor.matmul(ps_q[:], lhsT=ones[:],
                                     rhs=brow[:, 0:256], start=True, stop=False)
                    nc.tensor.matmul(ps_q[:], lhsT=xs[:, ei],
                                     rhs=wt[:, 0:256], start=False, stop=True)
                    nc.tensor.matmul(ps_kv[:], lhsT=ones[:],
                                     rhs=brow[:, 256:768], start=True, stop=False)
                    nc.tensor.matmul(ps_kv[:], lhsT=xt[:, ei],
                                     rhs=wt[:, 256:768], start=False, stop=True)
                    # PSUM->SBUF fp16 cast on the scalar engine (off DVE)
                    o = i * 768
                    nc.scalar.activation(
                        out=qkv[:, o:o + 256], in_=ps_q[:],
                        func=mybir.ActivationFunctionType.Copy)
                    nc.scalar.activation(
                        out=qkv[:, o + 256:o + 768], in_=ps_kv[:],
                        func=mybir.ActivationFunctionType.Copy)

                # scores: prod[t,h,g,d] = Q[t,h,d] * K[t,g,d]
                # (ISA allows max 3 free dims -> one TT per 128-edge tile)
                prod = wp.tile([128, C * 2048], f16, tag="prod")
                for i in range(C):
                    o = i * 768
                    qa = (qkv[:, o:o + 256]
                          .rearrange("p (h d) -> p h d", h=H)
                          .unsqueeze(2).to_broadcast([128, H, H, D]))
                    ka = (qkv[:, o + 256:o + 512]
                          .rearrange("p (g d) -> p g d", g=H)
                          .unsqueeze(1).to_broadcast([128, H, H, D]))
                    nc.vector.tensor_tensor(
                        out=prod[:, i * 2048:(i + 1) * 2048]
                            .rearrange("p (h g d) -> p h g d", h=H, g=H),
                        in0=qa, in1=ka, op=MUL)
                # d-sum via packed TT add-tree: TENSOR_REDUCE runs at
                # 1 elem/cycle while packed fp16 TT adds hit the 2x mode
                scores = wp.tile([128, C * 64], f16, tag="scores")
                t16 = wp.tile([128, C * 1024], f16, tag="t16")
                t8 = wp.tile([128, C * 512], f16, tag="t8")
                t4 = wp.tile([128, C * 256], f16, tag="t4")
                t2 = wp.tile([128, C * 128], f16, tag="t2")
                with nc.allow_low_precision(reason="fp16 add tree"):
                    for dst, src_, w in ((t16, prod, 32), (t8, t16, 16),
                                         (t4, t8, 8), (t2, t4, 4),
                                         (scores, t2, 2)):
                        v = src_[:].rearrange("p (a w) -> p a w", w=w)
                        nc.vector.tensor_tensor(
                            out=dst[:].rearrange("p (a w) -> p a w", w=w // 2),
                            in0=v[:, :, 0:w // 2], in1=v[:, :, w // 2:w],
                            op=ADD)
                u = wp.tile([128, C * 64], f32, tag="u")
                nc.scalar.activation(out=u[:], in_=scores[:],
                                     func=mybir.ActivationFunctionType.Exp,
                                     scale=float(1.0 / np.sqrt(D)))
                ssum = wp.tile([128, C * 8], f32, tag="ssum")
                rinv = wp.tile([128, C * 8], f32, tag="rinv")
                nc.vector.tensor_reduce(
                    out=ssum[:],
                    in_=u[:].rearrange("p (a g) -> p a g", g=H),
                    axis=X, op=ADD)
                nc.vector.reciprocal_approx_fast(out=rinv[:], in_=ssum[:])
                attn = wp.tile([128, C * 64], f16, tag="attn")
                with nc.allow_low_precision(reason="fp16 attn"):
                    nc.vector.tensor_tensor(
                        out=attn[:].rearrange("p (a g) -> p a g", g=H),
                        in0=u[:].rearrange("p (a g) -> p a g", g=H),
                        in1=rinv[:].rearrange("p (a o) -> p a o", o=1)
                            .to_broadcast([128, C * 8, H]),
                        op=MUL)
                # msg[t,h,d] = sum_g attn[t,h,g] * V[t,d,g]  (V host-permuted)
                prod2 = wp.tile([128, C * 2048], f16, tag="prod")
                for i in range(C):
                    aa = (attn[:, i * 64:(i + 1) * 64]
                          .rearrange("p (h g) -> p h g", h=H)
                          .unsqueeze(2).to_broadcast([128, H, D, H]))
                    va = (qkv[:, i * 768 + 512:(i + 1) * 768]
                          .rearrange("p (d g) -> p d g", d=D)
                          .unsqueeze(1).to_broadcast([128, H, D, H]))
                    nc.vector.tensor_tensor(
                        out=prod2[:, i * 2048:(i + 1) * 2048]
                            .rearrange("p (h d g) -> p h d g", h=H, d=D),
                        in0=aa, in1=va, op=MUL)
                # g-sum via packed fp16 TT add-tree (g=8 -> 3 levels).
                # NOTE: folding the last level into extra segment matmuls
                # (strided lhsT) was tried and REGRESSED 13%: strided
                # LDWEIGHTS is ~1.6x slower and the added PE SBUF traffic
                # taxes concurrent DVE TTs ~20% (SBUF bandwidth coupling).
                msg = wp.tile([128, C * 256], f16, tag="msg")
                q4 = wp.tile([128, C * 1024], f16, tag="t16")
                q2 = wp.tile([128, C * 512], f16, tag="t8")
                with nc.allow_low_precision(reason="fp16 add tree"):
                    for dst, src_, w in ((q4, prod2, 8), (q2, q4, 4),
                                         (msg, q2, 2)):
                        v = src_[:].rearrange("p (a w) -> p a w", w=w)
                        nc.vector.tensor_tensor(
                            out=dst[:].rearrange("p (a w) -> p a w", w=w // 2),
                            in0=v[:, :, 0:w // 2], in1=v[:, :, w // 2:w],
                            op=ADD)

                # segment sum: aggT = msg_chunk^T @ S  (accumulate over chunks)
                agg1 = pa.tile([128, 128], f32, tag="agg1")
                agg2 = pa.tile([128, 128], f32, tag="agg2")
                agg3 = pa.tile([4, 128], f32, tag="agg3")
                for i in range(C):
                    st, sp = (i == 0), (i == C - 1)
                    s_i = sb[:, i * 128:(i + 1) * 128]
                    nc.tensor.matmul(agg1[:], lhsT=msg[:, i * 256:i * 256 + 128],
                                     rhs=s_i, start=st, stop=sp)
                    nc.tensor.matmul(agg2[:], lhsT=msg[:, i * 256 + 128:(i + 1) * 256],
                                     rhs=s_i, start=st, stop=sp)
                    t_ix = b * C + i
                    nc.tensor.matmul(agg3[:], lhsT=oh[:, t_ix * 4:(t_ix + 1) * 4],
                                     rhs=s_i, start=st, stop=sp)
                a1 = wp.tile([128, 128], bf16, tag="a1")
                nc.scalar.activation(out=a1[:], in_=agg1[:],
                                     func=mybir.ActivationFunctionType.Copy)
                a2 = wp.tile([128, 128], bf16, tag="a2")
                nc.scalar.activation(out=a2[:], in_=agg2[:],
                                     func=mybir.ActivationFunctionType.Copy)
                a3 = wp.tile([4, 128], bf16, tag="a3")
                nc.scalar.activation(out=a3[:], in_=agg3[:],
                                     func=mybir.ActivationFunctionType.Copy)
                mt = pa.tile([128, 128], f32, tag="mt")
                nc.tensor.matmul(mt[:], lhsT=w2ab[:, 0:128], rhs=a1[:],
                                 start=True, stop=False)
                nc.tensor.matmul(mt[:], lhsT=w2ab[:, 128:256], rhs=a2[:],
                                 start=False, stop=False)
                nc.tensor.matmul(mt[:], lhsT=w2c[:], rhs=a3[:],
                                 start=False, stop=True)
                nc.scalar.activation(out=outsb[:, b * 128:(b + 1) * 128],
                                     in_=mt[:],
                                     func=mybir.ActivationFunctionType.Copy)

            nc.sync.dma_start(out=out[:, :], in_=outsb[:])
    return nc


def kernel(node_features, edges, edge_types, Wq, bq, Wk, bk, Wv, bv,
           edge_emb, Wo, bo):
    x = np.asarray(node_features, dtype=np.float32)
    edges = np.asarray(edges, dtype=np.int64)
    et = np.asarray(edge_types, dtype=np.int64)
    Wq = np.asarray(Wq, np.float32); bq = np.asarray(bq, np.float32)
    Wk = np.asarray(Wk, np.float32); bk = np.asarray(bk, np.float32)
    Wv = np.asarray(Wv, np.float32); bv = np.asarray(bv, np.float32)
    edge_emb = np.asarray(edge_emb, np.float32)
    Wo = np.asarray(Wo, np.float32); bo = np.asarray(bo, np.float32)

    src, tgt = edges[:, 0], edges[:, 1]
    n_nodes = x.shape[0]
    deg = np.bincount(src, minlength=n_nodes)

    # Pack nodes into blocks (<=128 nodes, <=C*128 edges each), uniform
    # across cores so one SPMD program serves all 8. LPT-greedy; bump the
    # block count if packing is tight.
    members = None
    for NB in (50, 51, 52, 56):
        B = NB * N_CORES
        members = _pack_blocks(deg, B, ecap=768)
        if members is not None:
            break
    if members is None:  # degenerate degree distribution: fall back
        NB = int(np.ceil(n_nodes / 128 / N_CORES)) + 8
        B = NB * N_CORES
        members = _pack_blocks(deg, B, ecap=10**9)
    loads = np.array([int(deg[m].sum()) for m in members])
    C = max(1, int(np.ceil(loads.max() / 128)))
    T = NB * C
    E_pad = T * 128
    NODES_PAD = NB * 128

    # snake-deal blocks (sorted by load desc) to cores for edge balance
    order = np.argsort(-loads, kind="stable")
    core_blocks = [[] for _ in range(N_CORES)]
    for k, bid in enumerate(order):
        rnd, pos = divmod(k, N_CORES)
        c = pos if rnd % 2 == 0 else N_CORES - 1 - pos
        core_blocks[c].append(bid)

    # node -> (core, local block index, slot) lookup tables
    node_core = np.empty(n_nodes, np.int64)
    node_blk = np.empty(n_nodes, np.int64)
    node_slot = np.empty(n_nodes, np.int64)
    for c in range(N_CORES):
        for bi, bid in enumerate(core_blocks[c]):
            ids = np.asarray(members[bid], dtype=np.int64)
            node_core[ids] = c
            node_blk[ids] = bi
            node_slot[ids] = np.arange(len(ids))

    # shared weight blocks
    Wvp = Wv.reshape(H, D, IN_DIM).transpose(1, 0, 2).reshape(HID, IN_DIM)
    bvp = bv.reshape(H, D).T.reshape(HID)
    Wt_np = np.concatenate([Wq.T, Wk.T, Wvp.T], axis=1).astype(BF16)
    b_t = np.concatenate([bq, bk, bvp]).astype(np.float32)
    Brow_np = b_t[None, :].astype(BF16)
    Ones_np = np.ones((1, 128), dtype=BF16)
    WoT = Wo.T.astype(np.float32)
    # W2ab[:, 0:128] = WoT rows 0:128 ; W2ab[:, 128:256] = WoT rows 128:256
    W2ab_np = np.concatenate([WoT[0:128], WoT[128:256]], axis=1).astype(BF16)
    W2c_np = np.concatenate([edge_emb @ Wo.T, bo[None, :]], axis=0).astype(BF16)

    e_core = node_core[src]
    e_blk = node_blk[src]
    e_slot = node_slot[src]

    in_maps = []
    for c in range(N_CORES):
        idx = np.nonzero(e_core == c)[0]
        blk = e_blk[idx]
        order = np.argsort(blk, kind="stable")
        idx, blk = idx[order], blk[order]
        counts = np.bincount(blk, minlength=NB)
        starts = np.zeros(NB, np.int64)
        starts[1:] = np.cumsum(counts)[:-1]
        within = np.arange(len(idx)) - starts[blk]
        pos = blk * (C * 128) + within
        s_loc = e_slot[idx]

        xs_full = np.zeros((E_pad, IN_DIM), np.float32)
        xs_full[pos] = x[src[idx]]
        xt_full = np.zeros((E_pad, IN_DIM), np.float32)
        xt_full[pos] = x[tgt[idx]]
        S_full = np.zeros((E_pad, 128), np.float32)
        S_full[pos, s_loc] = 1.0
        OH_full = np.zeros((E_pad, 4), np.float32)
        OH_full[pos, et[idx]] = 1.0
        OH_full[pos, 3] = 1.0

        in_maps.append({
            "xsT": np.ascontiguousarray(xs_full.T).astype(BF16),
            "xtT": np.ascontiguousarray(xt_full.T).astype(BF16),
            "S2": np.ascontiguousarray(
                S_full.reshape(T, 128, 128).transpose(1, 0, 2)
                .reshape(128, T * 128)).astype(np.float16),
            "OHt": np.ascontiguousarray(
                OH_full.reshape(T, 128, 4).transpose(1, 0, 2)
                .reshape(128, T * 4)).astype(np.float16),
            "Wt": Wt_np, "Brow": Brow_np, "Ones": Ones_np,
            "W2ab": W2ab_np, "W2c": W2c_np,
        })

    if (NB, C) not in _prog_cache:
        nc = _build_program(NB, C)
        nc.finalize()  # runs Bacc passes incl. sync-wait legalization
        _prog_cache[(NB, C)] = nc
    nc = _prog_cache[(NB, C)]

    kw = {}
    tr = os.environ.get("KERNEL_TRACE_DIR")
    if tr:
        kw = dict(trace=True, tmpdir=tr)
    res = run_bass_kernel_spmd(nc, in_maps, core_ids=list(range(N_CORES)), **kw)
    global LAST_RESULTS
    LAST_RESULTS = res
    outs = res.results

    messages = np.zeros((n_nodes, IN_DIM), np.float32)
    for c in range(N_CORES):
        o = np.asarray(outs[c]["out"], dtype=np.float32)  # [128, NODES_PAD]
        slot_ids = np.full(NODES_PAD, -1, np.int64)
        for bi, bid in enumerate(core_blocks[c]):
            ids = np.asarray(members[bid], dtype=np.int64)
            slot_ids[bi * 128:bi * 128 + len(ids)] = ids
        valid = slot_ids >= 0
        messages[slot_ids[valid]] = o[:, valid].T
    return messages



# revision 37
# speedup vs baseline: 1.2049x; 1.2049x over previous
"""GNN message-passing kernel for Trainium2 (8 NeuronCores, edge-parallel).

Strategy: LPT bin-pack nodes into 400 blocks (<=128 nodes, <=768 edges
each -> uniform 6 edge-chunks/block), snake-deal blocks to the 8 cores by
edge load. Each core owns its blocks' edges; outputs are disjoint -> no
collective; the host scatters block slots back to node ids.

Device pipeline per core (50 blocks x 6 chunks of 128 edges):
  1. PE: per-edge Q/K/V projections; bias comes free via a rank-1 PSUM
     seed (ones^T x bias-row, start=True) that the projection accumulates
     onto. K and V share lhsT=xt -> fused f=512 matmul.
  2. ACT: PSUM->SBUF fp16 casts (scalar engine, keeps DVE free).
  3. DVE: per-edge 8x8 head attention entirely in packed fp16 so every
     tensor_tensor hits the 2x_1p mode; d-sums and g-sums use TT add-trees
     (TENSOR_REDUCE is stuck at 1 elem/cycle; packed TT adds are 2x).
     exp on ACT (u stays f32 for range), reciprocal_approx_fast for 1/sum.
  4. PE: segment-sum via block one-hot matmul (S matrix from host),
     plus edge-type one-hot/count columns for the embedding & bias terms.
  5. PE: final output projection [Wo^T | emb@Wo^T | bo] per node block.
"""

import os
import sys

sys.path.insert(0, "/opt/trn_rl_repo")

import numpy as np
import ml_dtypes

from concourse import bass, bacc, mybir
import concourse.tile as tile
from concourse.bass_utils import run_bass_kernel_spmd

N_NODES = 50000
N_CORES = 8
IN_DIM = 128
HID = 256
H = 8
D = 32

BF16 = ml_dtypes.bfloat16
GP_MOD = 10**9     # GpSimd offload disabled: SBUF contention slows DVE
_prog_cache = {}
LAST_RESULTS = None


def _pack_blocks(deg, n_blocks, ecap, ncap=128):
    """LPT-pack nodes into n_blocks bins: <=ncap nodes, <=ecap edges each.
    Returns list of node-id lists, or None if infeasible."""
    import heapq
    order = np.argsort(-deg, kind="stable")
    heap = [(0, 0, b) for b in range(n_blocks)]
    heapq.heapify(heap)
    members = [[] for _ in range(n_blocks)]
    for n in order:
        d = int(deg[n])
        popped = []
        ok = False
        while heap:
            l, c, b = heapq.heappop(heap)
            if c < ncap and l + d <= ecap:
                members[b].append(n)
                heapq.heappush(heap, (l + d, c + 1, b))
                ok = True
                break
            popped.append((l, c, b))
        for p in popped:
            heapq.heappush(heap, p)
        if not ok:
            return None
    return members


def _build_program(NB, C):
    """C = edge-chunks (of 128) per node block; T = NB*C tiles per core."""
    NODES_PAD = NB * 128
    T = NB * C
    E_pad = T * 128
    f32, bf16 = mybir.dt.float32, mybir.dt.bfloat16
    f16 = mybir.dt.float16
    X = mybir.AxisListType.X
    MUL, ADD = mybir.AluOpType.mult, mybir.AluOpType.add

    nc = bacc.Bacc("TRN2", target_bir_lowering=False)
    xsT = nc.dram_tensor("xsT", [128, E_pad], bf16, kind="ExternalInput")
    xtT = nc.dram_tensor("xtT", [128, E_pad], bf16, kind="ExternalInput")
    S2 = nc.dram_tensor("S2", [128, T * 128], f16, kind="ExternalInput")
    OHt = nc.dram_tensor("OHt", [128, T * 4], f16, kind="ExternalInput")
    Wt = nc.dram_tensor("Wt", [128, 768], bf16, kind="ExternalInput")
    Brow = nc.dram_tensor("Brow", [1, 768], bf16, kind="ExternalInput")
    Ones = nc.dram_tensor("Ones", [1, 128], bf16, kind="ExternalInput")
    W2ab = nc.dram_tensor("W2ab", [128, 256], bf16, kind="ExternalInput")
    W2c = nc.dram_tensor("W2c", [4, 128], bf16, kind="ExternalInput")
    out = nc.dram_tensor("out", [128, NODES_PAD], f32, kind="ExternalOutput")

    with tile.TileContext(nc) as tc:
        with tc.tile_pool(name="const", bufs=1) as cp, \
             tc.tile_pool(name="io", bufs=2) as iop, \
             tc.tile_pool(name="work", bufs=2) as wp, \
             tc.tile_pool(name="pproj", bufs=1, space="PSUM") as pp, \
             tc.tile_pool(name="pacc", bufs=1, space="PSUM") as pa:

            wt = cp.tile([128, 768], bf16)
            nc.sync.dma_start(out=wt[:], in_=Wt[:, :])
            brow = cp.tile([1, 768], bf16)
            nc.sync.dma_start(out=brow[:], in_=Brow[:, :])
            ones = cp.tile([1, 128], bf16)
            nc.sync.dma_start(out=ones[:], in_=Ones[:, :])
            oh = cp.tile([128, T * 4], f16)
            w2ab = cp.tile([128, 256], bf16)
            w2c = cp.tile([4, 128], bf16)
            outsb = cp.tile([128, NODES_PAD], f32)

            for b in range(NB):
                esl = slice(b * C * 128, (b + 1) * C * 128)
                xs = iop.tile([128, C * 128], bf16, tag="xs")
                nc.sync.dma_start(out=xs[:], in_=xsT[:, esl])
                xt = iop.tile([128, C * 128], bf16, tag="xt")
                nc.sync.dma_start(out=xt[:], in_=xtT[:, esl])
                sb = iop.tile([128, C * 128], f16, tag="sb")
                nc.sync.dma_start(out=sb[:], in_=S2[:, esl])
                if b == 0:
                    # issued after block 0's inputs so the 300KB+ one-hot
                    # table doesn't delay the pipeline head (only needed at
                    # each block's segment stage)
                    nc.sync.dma_start(out=oh[:], in_=OHt[:, :])
                    nc.sync.dma_start(out=w2ab[:], in_=W2ab[:, :])
                    nc.sync.dma_start(out=w2c[:], in_=W2c[:, :])

                qkv = wp.tile([128, C * 768], f16, tag="qkv")
                for i in range(C):
                    ps_q = pp.tile([128, 256], f32, tag="psq")
                    ps_kv = pp.tile([128, 512], f32, tag="pskv")
                    ei = slice(i * 128, (i + 1) * 128)
                    # bias via rank-1 PSUM seed (ones^T x brow), then the
                    # projection accumulates on top -> bias-add is free on
                    # PE; K and V share lhsT=xt so they fuse into one f=512
                    # matmul per stage
                    nc.tensor.matmul(ps_q[:], lhsT=ones[:],
                                     rhs=brow[:, 0:256], start=True, stop=False)
                    nc.tensor.matmul(ps_q[:], lhsT=xs[:, ei],
                                     rhs=wt[:, 0:256], start=False, stop=True)
                    nc.tensor.matmul(ps_kv[:], lhsT=ones[:],
                                     rhs=brow[:, 256:768], start=True, stop=False)
                    nc.tensor.matmul(ps_kv[:], lhsT=xt[:, ei],
                                     rhs=wt[:, 256:768], start=False, stop=True)
                    # PSUM->SBUF fp16 cast on the scalar engine (off DVE)
                    o = i * 768
                    nc.scalar.activation(
                        out=qkv[:, o:o + 256], in_=ps_q[:],
                        func=mybir.ActivationFunctionType.Copy)
                    nc.scalar.activation(
                        out=qkv[:, o + 256:o + 768], in_=ps_kv[:],
                        func=mybir.ActivationFunctionType.Copy)

                # scores: prod[t,h,g,d] = Q[t,h,d] * K[t,g,d]
                # (ISA allows max 3 free dims -> one TT per 128-edge tile)
                prod = wp.tile([128, C * 2048], f16, tag="prod")
                for i in range(C):
                    o = i * 768
                    qa = (qkv[:, o:o + 256]
                          .rearrange("p (h d) -> p h d", h=H)
                          .unsqueeze(2).to_broadcast([128, H, H, D]))
                    ka = (qkv[:, o + 256:o + 512]
                          .rearrange("p (g d) -> p g d", g=H)
                          .unsqueeze(1).to_broadcast([128, H, H, D]))
                    nc.vector.tensor_tensor(
                        out=prod[:, i * 2048:(i + 1) * 2048]
                            .rearrange("p (h g d) -> p h g d", h=H, g=H),
                        in0=qa, in1=ka, op=MUL)
                # d-sum via packed TT add-tree: TENSOR_REDUCE runs at
                # 1 elem/cycle while packed fp16 TT adds hit the 2x mode
                scores = wp.tile([128, C * 64], f16, tag="scores")
                t16 = wp.tile([128, C * 1024], f16, tag="t16")
                t8 = wp.tile([128, C * 512], f16, tag="t8")
                t4 = wp.tile([128, C * 256], f16, tag="t4")
                t2 = wp.tile([128, C * 128], f16, tag="t2")
                with nc.allow_low_precision(reason="fp16 add tree"):
                    for dst, src_, w in ((t16, prod, 32), (t8, t16, 16),
                                         (t4, t8, 8), (t2, t4, 4),
                                         (scores, t2, 2)):
                        v = src_[:].rearrange("p (a w) -> p a w", w=w)
                        nc.vector.tensor_tensor(
                            out=dst[:].rearrange("p (a w) -> p a w", w=w // 2),
                            in0=v[:, :, 0:w // 2], in1=v[:, :, w // 2:w],
                            op=ADD)
                u = wp.tile([128, C * 64], f32, tag="u")
                nc.scalar.activation(out=u[:], in_=scores[:],
                                     func=mybir.ActivationFunctionType.Exp,
                                     scale=float(1.0 / np.sqrt(D)))
                ssum = wp.tile([128, C * 8], f32, tag="ssum")
                rinv = wp.tile([128, C * 8], f32, tag="rinv")
                nc.vector.tensor_reduce(
                    out=ssum[:],
                    in_=u[:].rearrange("p (a g) -> p a g", g=H),
                    axis=X, op=ADD)
                nc.vector.reciprocal_approx_fast(out=rinv[:], in_=ssum[:])
                attn = wp.tile([128, C * 64], f16, tag="attn")
                with nc.allow_low_precision(reason="fp16 attn"):
                    nc.vector.tensor_tensor(
                        out=attn[:].rearrange("p (a g) -> p a g", g=H),
                        in0=u[:].rearrange("p (a g) -> p a g", g=H),
                        in1=rinv[:].rearrange("p (a o) -> p a o", o=1)
                            .to_broadcast([128, C * 8, H]),
                        op=MUL)
                # msg[t,h,d] = sum_g attn[t,h,g] * V[t,d,g]  (V host-permuted)
                prod2 = wp.tile([128, C * 2048], f16, tag="prod")
                for i in range(C):
                    aa = (attn[:, i * 64:(i + 1) * 64]
                          .rearrange("p (h g) -> p h g", h=H)
                          .unsqueeze(2).to_broadcast([128, H, D, H]))
                    va = (qkv[:, i * 768 + 512:(i + 1) * 768]
                          .rearrange("p (d g) -> p d g", d=D)
                          .unsqueeze(1).to_broadcast([128, H, D, H]))
                    nc.vector.tensor_tensor(
                        out=prod2[:, i * 2048:(i + 1) * 2048]
                            .rearrange("p (h d g) -> p h d g", h=H, d=D),
                        in0=aa, in1=va, op=MUL)
                # g-sum via packed fp16 TT add-tree (g=8 -> 3 levels).
                # NOTE: folding the last level into extra segment matmuls
                # (strided lhsT) was tried and REGRESSED 13%: strided
                # LDWEIGHTS is ~1.6x slower and the added PE SBUF traffic
                # taxes concurrent DVE TTs ~20% (SBUF bandwidth coupling).
                msg = wp.tile([128, C * 256], f16, tag="msg")
                q4 = wp.tile([128, C * 1024], f16, tag="t16")
                q2 = wp.tile([128, C * 512], f16, tag="t8")
                with nc.allow_low_precision(reason="fp16 add tree"):
                    for dst, src_, w in ((q4, prod2, 8), (q2, q4, 4),
                                         (msg, q2, 2)):
                        v = src_[:].rearrange("p (a w) -> p a w", w=w)
                        nc.vector.tensor_tensor(
                            out=dst[:].rearrange("p (a w) -> p a w", w=w // 2),
                            in0=v[:, :, 0:w // 2], in1=v[:, :, w // 2:w],
                            op=ADD)

                # segment sum: aggT = msg_chunk^T @ S  (accumulate over chunks)
                agg1 = pa.tile([128, 128], f32, tag="agg1")
                agg2 = pa.tile([128, 128], f32, tag="agg2")
                agg3 = pa.tile([4, 128], f32, tag="agg3")
                for i in range(C):
                    st, sp = (i == 0), (i == C - 1)
                    s_i = sb[:, i * 128:(i + 1) * 128]
                    nc.tensor.matmul(agg1[:], lhsT=msg[:, i * 256:i * 256 + 128],
                                     rhs=s_i, start=st, stop=sp)
                    nc.tensor.matmul(agg2[:], lhsT=msg[:, i * 256 + 128:(i + 1) * 256],
                                     rhs=s_i, start=st, stop=sp)
                    t_ix = b * C + i
                    nc.tensor.matmul(agg3[:], lhsT=oh[:, t_ix * 4:(t_ix + 1) * 4],
                                     rhs=s_i, start=st, stop=sp)
                a1 = wp.tile([128, 128], bf16, tag="a1")
                nc.scalar.activation(out=a1[:], in_=agg1[:],
                                     func=mybir.ActivationFunctionType.Copy)
                a2 = wp.tile([128, 128], bf16, tag="a2")
                nc.scalar.activation(out=a2[:], in_=agg2[:],
                                     func=mybir.ActivationFunctionType.Copy)
                a3 = wp.tile([4, 128], bf16, tag="a3")
                nc.scalar.activation(out=a3[:], in_=agg3[:],
                                     func=mybir.ActivationFunctionType.Copy)
                mt = pa.tile([128, 128], f32, tag="mt")
                nc.tensor.matmul(mt[:], lhsT=w2ab[:, 0:128], rhs=a1[:],
                                 start=True, stop=False)
                nc.tensor.matmul(mt[:], lhsT=w2ab[:, 128:256], rhs=a2[:],
                                 start=False, stop=False)
                nc.tensor.matmul(mt[:], lhsT=w2c[:], rhs=a3[:],
                                 start=False, stop=True)
                nc.scalar.activation(out=outsb[:, b * 128:(b + 1) * 128],
                                     in_=mt[:],
                                     func=mybir.ActivationFunctionType.Copy)
                # per-block output DMA overlaps with later blocks' compute
                # (a single end-of-kernel 3.3MB DMA added a ~13us idle tail)
                nc.sync.dma_start(out=out[:, b * 128:(b + 1) * 128],
                                  in_=outsb[:, b * 128:(b + 1) * 128])
    return nc


def kernel(node_features, edges, edge_types, Wq, bq, Wk, bk, Wv, bv,
           edge_emb, Wo, bo):
    x = np.asarray(node_features, dtype=np.float32)
    edges = np.asarray(edges, dtype=np.int64)
    et = np.asarray(edge_types, dtype=np.int64)
    Wq = np.asarray(Wq, np.float32); bq = np.asarray(bq, np.float32)
    Wk = np.asarray(Wk, np.float32); bk = np.asarray(bk, np.float32)
    Wv = np.asarray(Wv, np.float32); bv = np.asarray(bv, np.float32)
    edge_emb = np.asarray(edge_emb, np.float32)
    Wo = np.asarray(Wo, np.float32); bo = np.asarray(bo, np.float32)

    src, tgt = edges[:, 0], edges[:, 1]
    n_nodes = x.shape[0]
    deg = np.bincount(src, minlength=n_nodes)

    # Pack nodes into blocks (<=128 nodes, <=C*128 edges each), uniform
    # across cores so one SPMD program serves all 8. LPT-greedy; bump the
    # block count if packing is tight.
    members = None
    for NB in (50, 51, 52, 56):
        B = NB * N_CORES
        members = _pack_blocks(deg, B, ecap=768)
        if members is not None:
            break
    if members is None:  # degenerate degree distribution: fall back
        NB = int(np.ceil(n_nodes / 128 / N_CORES)) + 8
        B = NB * N_CORES
        members = _pack_blocks(deg, B, ecap=10**9)
    loads = np.array([int(deg[m].sum()) for m in members])
    C = max(1, int(np.ceil(loads.max() / 128)))
    T = NB * C
    E_pad = T * 128
    NODES_PAD = NB * 128

    # snake-deal blocks (sorted by load desc) to cores for edge balance
    order = np.argsort(-loads, kind="stable")
    core_blocks = [[] for _ in range(N_CORES)]
    for k, bid in enumerate(order):
        rnd, pos = divmod(k, N_CORES)
        c = pos if rnd % 2 == 0 else N_CORES - 1 - pos
        core_blocks[c].append(bid)

    # node -> (core, local block index, slot) lookup tables
    node_core = np.empty(n_nodes, np.int64)
    node_blk = np.empty(n_nodes, np.int64)
    node_slot = np.empty(n_nodes, np.int64)
    for c in range(N_CORES):
        for bi, bid in enumerate(core_blocks[c]):
            ids = np.asarray(members[bid], dtype=np.int64)
            node_core[ids] = c
            node_blk[ids] = bi
            node_slot[ids] = np.arange(len(ids))

    # shared weight blocks
    Wvp = Wv.reshape(H, D, IN_DIM).transpose(1, 0, 2).reshape(HID, IN_DIM)
    bvp = bv.reshape(H, D).T.reshape(HID)
    Wt_np = np.concatenate([Wq.T, Wk.T, Wvp.T], axis=1).astype(BF16)
    b_t = np.concatenate([bq, bk, bvp]).astype(np.float32)
    Brow_np = b_t[None, :].astype(BF16)
    Ones_np = np.ones((1, 128), dtype=BF16)
    WoT = Wo.T.astype(np.float32)
    # W2ab[:, 0:128] = WoT rows 0:128 ; W2ab[:, 128:256] = WoT rows 128:256
    W2ab_np = np.concatenate([WoT[0:128], WoT[128:256]], axis=1).astype(BF16)
    W2c_np = np.concatenate([edge_emb @ Wo.T, bo[None, :]], axis=0).astype(BF16)

    e_core = node_core[src]
    e_blk = node_blk[src]
    e_slot = node_slot[src]

    in_maps = []
    for c in range(N_CORES):
        idx = np.nonzero(e_core == c)[0]
        blk = e_blk[idx]
        order = np.argsort(blk, kind="stable")
        idx, blk = idx[order], blk[order]
        counts = np.bincount(blk, minlength=NB)
        starts = np.zeros(NB, np.int64)
        starts[1:] = np.cumsum(counts)[:-1]
        within = np.arange(len(idx)) - starts[blk]
        pos = blk * (C * 128) + within
        s_loc = e_slot[idx]

        xs_full = np.zeros((E_pad, IN_DIM), np.float32)
        xs_full[pos] = x[src[idx]]
        xt_full = np.zeros((E_pad, IN_DIM), np.float32)
        xt_full[pos] = x[tgt[idx]]
        S_full = np.zeros((E_pad, 128), np.float32)
        S_full[pos, s_loc] = 1.0
        OH_full = np.zeros((E_pad, 4), np.float32)
        OH_full[pos, et[idx]] = 1.0
        OH_full[pos, 3] = 1.0

        in_maps.append({
            "xsT": np.ascontiguousarray(xs_full.T).astype(BF16),
            "xtT": np.ascontiguousarray(xt_full.T).astype(BF16),
            "S2": np.ascontiguousarray(
                S_full.reshape(T, 128, 128).transpose(1, 0, 2)
                .reshape(128, T * 128)).astype(np.float16),
            "OHt": np.ascontiguousarray(
                OH_full.reshape(T, 128, 4).transpose(1, 0, 2)
                .reshape(128, T * 4)).astype(np.float16),
            "Wt": Wt_np, "Brow": Brow_np, "Ones": Ones_np,
            "W2ab": W2ab_np, "W2c": W2c_np,
        })

    if (NB, C) not in _prog_cache:
        nc = _build_program(NB, C)
        nc.finalize()  # runs Bacc passes incl. sync-wait legalization
        _prog_cache[(NB, C)] = nc
    nc = _prog_cache[(NB, C)]

    kw = {}
    tr = os.environ.get("KERNEL_TRACE_DIR")
    if tr:
        kw = dict(trace=True, tmpdir=tr)
    res = run_bass_kernel_spmd(nc, in_maps, core_ids=list(range(N_CORES)), **kw)
    global LAST_RESULTS
    LAST_RESULTS = res
    outs = res.results

    messages = np.zeros((n_nodes, IN_DIM), np.float32)
    for c in range(N_CORES):
        o = np.asarray(outs[c]["out"], dtype=np.float32)  # [128, NODES_PAD]
        slot_ids = np.full(NODES_PAD, -1, np.int64)
        for bi, bid in enumerate(core_blocks[c]):
            ids = np.asarray(members[bid], dtype=np.int64)
            slot_ids[bi * 128:bi * 128 + len(ids)] = ids
        valid = slot_ids >= 0
        messages[slot_ids[valid]] = o[:, valid].T
    return messages

